# revision 1
# baseline (speedup 1.0000x reference)
"""Multi-head causal attention (B=4, S=2048, D=1024, H=16, RoPE) on 8 TRN2 cores.

Sharding: core = (batch b, head-group g of 8 heads).  Each core computes
qkv projection for its (b, g), RoPE, causal attention, and a partial
out-projection (contraction over its 512 head-dims).  Host sums the two
partials per batch.

Device layouts (per core):
  qk^T  [1024, S]   feature-major: rows 0:512 = q (8 heads x 64), 512:1024 = k
  v     [S, 520]    token-major, 65 cols/head: 64 dims + a ones column used to
                    accumulate softmax denominators during the P@V matmul
  S^T   [t, q]      scores transposed so softmax sums come out of the matmul
  O^T   [512, S]    per-head outputs, feature-major, ready as out-proj lhsT
"""

import math

import numpy as np

import concourse.bass as bass
import concourse.bacc as bacc
import concourse.mybir as mybir
from concourse import library_config, tile
from concourse.bass_utils import run_bass_kernel_spmd

AF = mybir.ActivationFunctionType
ALU = mybir.AluOpType
F32 = mybir.dt.float32

N_HEADS = 16
THETA = 10000.0
D = 1024
HD = 64
HL = 8          # heads per core
VW = HD + 1     # v columns per head (64 dims + ones)
NB = 512        # stage-1 token block
QB = 512        # query block
TT = 128        # key/value tile
MM_DT = mybir.dt.float32r   # matmul compute dtype (f32 data, single-pass PE)


def _host_constants(S):
    """RoPE tables, sign vector, causal masks (input-independent)."""
    half = HD // 2
    inv = 1.0 / (THETA ** (np.arange(half, dtype=np.float64) / half))
    t = np.arange(S, dtype=np.float64)
    ang = inv[:, None] * t[None, :]                      # [32, S]
    ropeC = np.tile(np.cos(ang), (4, 1)).astype(np.float32)   # [128, S]
    sinT = np.tile(np.sin(ang), (4, 1)).astype(np.float32)    # [128, S]
    sig = np.tile(np.r_[-np.ones(32), np.ones(32)], 2).astype(np.float32)[:, None]

    # masks2[p, oi*2*QB + h*QB + c] = 1.0 iff c >= p + 128*oi  (duplicated per head)
    p = np.arange(128)[:, None]
    c = np.arange(QB)[None, :]
    blocks = []
    for oi in range(QB // TT):
        m = (c >= p + TT * oi).astype(np.float32)
        blocks.append(np.concatenate([m, m], axis=1))    # duplicate for 2 heads
    masks2 = np.concatenate(blocks, axis=1)              # [128, 4*1024]
    return ropeC, sinT, sig, masks2


def build_nc(S=2048, mm_dt=MM_DT):
    nc = bacc.Bacc("TRN2", target_bir_lowering=False, debug=False)

    xT = nc.dram_tensor("xT", [D, S], F32, kind="ExternalInput").ap()
    wqkT = nc.dram_tensor("wqkT", [D, 2 * HL * HD], F32, kind="ExternalInput").ap()
    wvT = nc.dram_tensor("wvT", [D, HL * VW], F32, kind="ExternalInput").ap()
    woutT = nc.dram_tensor("woutT", [HL * HD, D], F32, kind="ExternalInput").ap()
    outp = nc.dram_tensor("outp", [S, D], F32, kind="ExternalOutput").ap()

    ropeC_np, sinT_np, sig_np, masks2_np = _host_constants(S)
    ropeC_d = nc.inline_tensor(ropeC_np, "ropeC").ap()
    sinT_d = nc.inline_tensor(sinT_np, "sinT").ap()
    sig_d = nc.inline_tensor(sig_np, "sig").ap()
    ones_d = nc.inline_tensor(np.ones((128, HL), dtype=np.float32), "ones8").ap()
    masks2_d = nc.inline_tensor(masks2_np, "masks2").ap()

    KD = D // 128        # 8 contraction tiles
    nNB = S // NB
    nQB = S // QB
    nMT = S // 128
    NTPB = QB // TT      # t-tiles per q-block (4)

    MMT = mm_dt   # dtype of every SBUF tile that feeds the PE

    def mm(ap):
        return ap

    # DRAM scratch for broadcasting softmax reciprocals across partitions
    recd = nc.dram_tensor("recd", [4 * nQB, 1024], F32).ap()

    with tile.TileContext(nc) as tc:
        with (
            tc.tile_pool(name="qk", bufs=1) as qk_pool,
            tc.tile_pool(name="vres", bufs=1) as v_pool,
        ):
            qk_sb = [qk_pool.tile([128, S], MMT, tag=f"qk{i}", name=f"qk{i}") for i in range(8)]
            v_sb = [v_pool.tile([128, HL * VW], MMT, tag=f"v{i}", name=f"v{i}") for i in range(nMT)]

            # ---------------- stage 1: qkv projection + RoPE ----------------
            with (
                tc.tile_pool(name="wqk", bufs=1) as wqk_pool,
                tc.tile_pool(name="wv", bufs=1) as wv_pool,
                tc.tile_pool(name="xs", bufs=14) as x_pool,
                tc.tile_pool(name="qs", bufs=3) as qs_pool,
                tc.tile_pool(name="raw", bufs=3) as raw_pool,
                tc.tile_pool(name="tm", bufs=3) as tm_pool,
                tc.tile_pool(name="tabs", bufs=1) as tab_pool,
                tc.tile_pool(name="ps_qk", bufs=3, space="PSUM") as psqk_pool,
                tc.tile_pool(name="ps_v", bufs=2, space="PSUM") as psv_pool,
            ):
                wqk_sb = [wqk_pool.tile([128, 2 * HL * HD], MMT, tag=f"wqk{k}", name=f"wqk{k}")
                          for k in range(KD)]
                wv_sb = [wv_pool.tile([128, HL * VW], MMT, tag=f"wv{k}", name=f"wv{k}")
                         for k in range(KD)]
                ropeC_sb = tab_pool.tile([128, S], F32, tag="ropeC")
                sinT_sb = tab_pool.tile([128, S], F32, tag="sinT")
                sig_sb = tab_pool.tile([128, 1], F32, tag="sig")
                def load_x(b):
                    tk = slice(b * NB, (b + 1) * NB)
                    xs = []
                    for k in range(KD):
                        xt = x_pool.tile([128, NB], MMT, name="xt")
                        nc.sync.dma_start(
                            xt[:], xT[k * 128:(k + 1) * 128, tk].bitcast(MMT))
                        xs.append(xt)
                    return xs

                # interleave first x block with weight loads so the first
                # matmul group starts after ~1 MiB of DMA, not ~8 MiB
                xts0 = []
                for k in range(KD):
                    nc.sync.dma_start(wqk_sb[k][:], wqkT[k * 128:(k + 1) * 128, :].bitcast(MMT))
                    xt = x_pool.tile([128, NB], MMT, name="xt")
                    nc.sync.dma_start(xt[:], xT[k * 128:(k + 1) * 128, 0:NB].bitcast(MMT))
                    xts0.append(xt)
                nc.sync.dma_start(ropeC_sb[:], ropeC_d[:])
                nc.sync.dma_start(sinT_sb[:], sinT_d[:])
                nc.sync.dma_start(sig_sb[:], sig_d[:])
                for k in range(KD):
                    nc.sync.dma_start(wv_sb[k][:], wvT[k * 128:(k + 1) * 128, :].bitcast(MMT))

                for nb in range(nNB):
                    tok = slice(nb * NB, (nb + 1) * NB)
                    xts = xts0 if nb == 0 else load_x(nb)

                    # q^T / k^T blocks (feature-major) + RoPE
                    for mo in range(8):
                        ps = psqk_pool.tile([128, NB], F32)
                        for k in range(KD):
                            nc.tensor.matmul(
                                ps[:],
                                lhsT=mm(wqk_sb[k][:, mo * 128:(mo + 1) * 128]),
                                rhs=mm(xts[k][:]),
                                start=(k == 0), stop=(k == KD - 1),
                            )
                        dst = qk_sb[mo][:, tok]
                        # dst = psum * cos-table
                        nc.vector.scalar_tensor_tensor(
                            dst, ps[:], 1.0, ropeC_sb[:, tok],
                            op0=ALU.mult, op1=ALU.mult,
                        )
                        # raw copy (DMA cannot read PSUM), then 32-row-swapped
                        # SBUF->SBUF DMA for the rotate-half operand
                        raw = raw_pool.tile([128, NB], F32, name="raw")
                        nc.scalar.copy(raw[:], ps[:])
                        qs = qs_pool.tile([128, NB], F32)
                        for b0 in (0, 64):
                            nc.scalar.dma_start(qs[b0:b0 + 32, :], raw[b0 + 32:b0 + 64, :])
                            nc.scalar.dma_start(qs[b0 + 32:b0 + 64, :], raw[b0:b0 + 32, :])
                        # tm = (qs * +-1) * sin-table ; dst += tm
                        tm = tm_pool.tile([128, NB], F32)
                        nc.vector.scalar_tensor_tensor(
                            tm[:], qs[:], sig_sb[:, 0:1], sinT_sb[:, tok],
                            op0=ALU.mult, op1=ALU.mult,
                        )
                        nc.vector.tensor_tensor(dst, dst, tm[:], op=ALU.add)

                    # v blocks (token-major, 65 cols/head)
                    for mt in range(NB // 128):
                        pv = psv_pool.tile([128, HL * VW], F32)
                        xsl = slice(mt * 128, (mt + 1) * 128)
                        for k in range(KD):
                            nc.tensor.matmul(
                                pv[:, 0:512],
                                lhsT=mm(xts[k][:, xsl]),
                                rhs=mm(wv_sb[k][:, 0:512]),
                                start=(k == 0), stop=(k == KD - 1),
                            )
                        for k in range(KD):
                            nc.tensor.matmul(
                                pv[:, 512:HL * VW],
                                lhsT=mm(xts[k][:, xsl]),
                                rhs=mm(wv_sb[k][:, 512:HL * VW]),
                                start=(k == 0), stop=(k == KD - 1),
                            )
                        vt = v_sb[nb * (NB // 128) + mt]
                        nc.vector.tensor_copy(vt[:], pv[:])
                        ones_ap = vt[:].rearrange("p (h c) -> p h c", h=HL)[:, :, HD]
                        nc.gpsimd.dma_start(ones_ap, ones_d.bitcast(MMT))

            # ---------------- stage 2: causal attention ----------------
            with tc.tile_pool(name="osb", bufs=1) as o_pool:
                o_sb = [o_pool.tile([128, S], MMT, tag=f"o{i}", name=f"o{i}") for i in range(4)]
                _stage2(nc, tc, mm, S, qk_sb, v_sb, o_sb, masks2_d, MMT, recd, woutT, outp)
    nc.compile()
    return nc


def _stage2(nc, tc, mm, S, qk_sb, v_sb, o_sb, masks2_d, MMT, recd, woutT, outp):
    nQB = S // QB
    NTPB = QB // TT
    with tc.tile_pool(name="wout", bufs=1) as wout_pool:
        wout_sb = [wout_pool.tile([128, D], MMT, tag=f"wo{i}", name=f"wo{i}")
                   for i in range(4)]
        for i in range(4):
            nc.sync.dma_start(wout_sb[i][:],
                              woutT[i * 128:(i + 1) * 128, :].bitcast(MMT))
        _stage2_body(nc, tc, mm, S, qk_sb, v_sb, o_sb, masks2_d, MMT, recd)
        _stage3(nc, tc, mm, S, o_sb, wout_sb, outp, MMT)


def _stage2_body(nc, tc, mm, S, qk_sb, v_sb, o_sb, masks2_d, MMT, recd):
    nQB = S // QB
    NTPB = QB // TT
    with (
        tc.tile_pool(name="masks", bufs=1) as m_pool,
        tc.tile_pool(name="sums", bufs=2) as sums_pool,
        tc.tile_pool(name="pp", bufs=4) as p_pool,
        tc.tile_pool(name="otmp", bufs=2) as otmp_pool,
        tc.tile_pool(name="ps_st", bufs=2, space="PSUM") as psst_pool,
        tc.tile_pool(name="ps_o", bufs=2, space="PSUM") as pso_pool,
    ):
        masks_sb = m_pool.tile([128, NTPB * 2 * QB], F32, tag="masks")
        nc.sync.dma_start(masks_sb[:], masks2_d[:])
        for hp in range(4):
            h0, h1 = 2 * hp, 2 * hp + 1
            qt = qk_sb[hp]
            kt = qk_sb[4 + hp]
            for qb in range(nQB):
                qsl = slice(qb * QB, (qb + 1) * QB)
                jmax = NTPB * qb + NTPB
                oaug = pso_pool.tile([VW, 1024], F32, name="oaug")
                for j in range(jmax):
                    oi = j - NTPB * qb
                    # first computed q column: trapezoid start, clamped so the
                    # matmul free dim stays >=256 (fp32r is 4 cyc/row below)
                    c0 = min(max(oi, 0) * TT, QB - 256)
                    st = psst_pool.tile([128, 1024], F32, name="st")
                    for hi in (0, 1):
                        base = hi * 64
                        nc.tensor.matmul(
                            st[:, hi * 512 + c0:(hi + 1) * 512],
                            lhsT=mm(kt[base:base + 64, j * TT:(j + 1) * TT]),
                            rhs=mm(qt[base:base + 64,
                                      qb * QB + c0:(qb + 1) * QB]),
                            start=True, stop=True,
                        )
                    pt = p_pool.tile([128, 1024], MMT, name="pt")
                    if c0 == 0:
                        nc.scalar.activation(pt[:], st[:], AF.Exp,
                                             scale=1.0 / math.sqrt(HD))
                    else:
                        for hi in (0, 1):
                            csl = slice(hi * 512 + c0, (hi + 1) * 512)
                            nc.scalar.activation(pt[:, csl], st[:, csl], AF.Exp,
                                                 scale=1.0 / math.sqrt(HD))
                    if oi >= 0:
                        for hi in (0, 1):
                            csl = slice(hi * 512 + c0, (hi + 1) * 512)
                            mslc = slice(oi * 2 * QB + hi * QB + c0,
                                         oi * 2 * QB + (hi + 1) * QB)
                            nc.vector.tensor_tensor(
                                pt[:, csl], pt[:, csl], masks_sb[:, mslc],
                                op=ALU.mult)
                    for hi, hh in ((0, h0), (1, h1)):
                        nc.tensor.matmul(
                            oaug[:, hi * 512 + c0:(hi + 1) * 512],
                            lhsT=mm(v_sb[j][:, VW * hh:VW * hh + VW]),
                            rhs=mm(pt[:, hi * 512 + c0:(hi + 1) * 512]),
                            start=(j == 0), stop=(j == jmax - 1),
                        )
                # softmax denominators live on partition 64 of oaug;
                # reciprocal in place on that partition (PSUM->SBUF), then
                # broadcast across 64 partitions via a DRAM bounce row
                rec = sums_pool.tile([VW, 1024], F32, name="rec", bufs=2)
                nc.vector.reciprocal(rec[HD:VW, :], oaug[HD:VW, :])
                srow = recd[hp * nQB + qb:hp * nQB + qb + 1, :]
                nc.sync.dma_start(srow, rec[HD:VW, :])
                bc = sums_pool.tile([64, 1024], F32, name="bc", bufs=2)
                nc.sync.dma_start(bc[:], srow.partition_broadcast(64))
                # even head: scale+extract straight into O^T rows 0:64
                nc.vector.tensor_tensor(
                    o_sb[hp][0:64, qsl], oaug[0:HD, 0:512],
                    bc[:, 0:512],
                    op=ALU.mult,
                )
                # odd head: scale at base 0, DMA into rows 64:128
                t2 = otmp_pool.tile([64, QB], MMT, name="t2")
                nc.vector.tensor_tensor(
                    t2[:], oaug[0:HD, 512:1024],
                    bc[:, 512:1024],
                    op=ALU.mult,
                )
                nc.sync.dma_start(o_sb[hp][64:128, qsl], t2[:])


def _stage3(nc, tc, mm, S, o_sb, wout_sb, outp, MMT):
    nMT = S // 128
    with (
        tc.tile_pool(name="outs", bufs=4) as out_pool,
        tc.tile_pool(name="ps_out", bufs=4, space="PSUM") as psout_pool,
    ):
        for mtt in range(nMT):
            tsl = slice(mtt * 128, (mtt + 1) * 128)
            for ib in range(2):
                po = psout_pool.tile([128, 512], F32, name="po")
                for hp in range(4):
                    nc.tensor.matmul(
                        po[:],
                        lhsT=mm(o_sb[hp][:, tsl]),
                        rhs=mm(wout_sb[hp][:, ib * 512:(ib + 1) * 512]),
                        start=(hp == 0), stop=(hp == 3),
                    )
                ot = out_pool.tile([128, 512], F32, name="ot")
                nc.scalar.copy(ot[:], po[:])
                nc.sync.dma_start(outp[tsl, ib * 512:(ib + 1) * 512], ot[:])


# ---------------------------------------------------------------------------
# host side
# ---------------------------------------------------------------------------

_cache = {}


def _get_nc(S):
    if S not in _cache:
        _cache[S] = build_nc(S)
    return _cache[S]


def _shard_weights(w_qkv, w_out, g):
    """Per-head-group weight shards in device layouts."""
    w_qkv = np.asarray(w_qkv, dtype=np.float32)
    w_out = np.asarray(w_out, dtype=np.float32)
    r = slice(g * 512, (g + 1) * 512)
    wq = w_qkv[0:D][r]            # [512, 1024]
    wk = w_qkv[D:2 * D][r]
    wv = w_qkv[2 * D:3 * D][r]
    wqkT = np.ascontiguousarray(np.concatenate([wq, wk], axis=0).T)   # [1024, 1024]
    wvT = np.zeros((D, HL * VW), dtype=np.float32)
    for hh in range(HL):
        wvT[:, VW * hh:VW * hh + HD] = wv[hh * HD:(hh + 1) * HD].T
    woutT = np.ascontiguousarray(w_out.T[r])   # [512, 1024]
    return wqkT, wvT, woutT


def kernel(x, w_qkv, w_out):
    x = np.asarray(x, dtype=np.float32)
    B, S, _D = x.shape
    assert _D == D
    nc = _get_nc(S)

    shards = [_shard_weights(w_qkv, w_out, g) for g in range(2)]
    in_maps = []
    for core in range(8):
        b, g = core // 2, core % 2
        wqkT, wvT, woutT = shards[g]
        in_maps.append({
            "xT": np.ascontiguousarray(x[b].T),
            "wqkT": wqkT,
            "wvT": wvT,
            "woutT": woutT,
        })
    res = run_bass_kernel_spmd(nc, in_maps, list(range(8)))
    out = np.empty((B, S, D), dtype=np.float32)
    for b in range(B):
        out[b] = res.results[2 * b]["outp"] + res.results[2 * b + 1]["outp"]
    return out



# revision 2
# speedup vs baseline: 1.2321x; 1.2321x over previous
"""Multi-head causal attention (B=4, S=2048, D=1024, H=16, RoPE) on 8 TRN2 cores.

Sharding: core = (batch b, head-group g of 8 heads).  Each core computes
qkv projection for its (b, g), RoPE, causal attention, and a partial
out-projection (contraction over its 512 head-dims).  Host sums the two
partials per batch.

v2 pipeline: qb-outer software pipeline interleaving stage1(nb+1) /
stage2(qb) / stage3(qb); bf16 data tiles; RoPE rotate-half via a PE
permutation matmul; softmax denominator broadcast via a PE outer
product; exact causal trapezoid (bf16 has no small-free-dim penalty).

Device layouts (per core):
  qk^T  [1024, S]  bf16 feature-major: tiles 0:4 = q (8 heads x 64), 4:8 = k
  v     [S, 520]   bf16 token-major, 65 cols/head: 64 dims + ones column
                   (accumulates softmax denominators during the P@V matmul)
  S^T   [t, q]     scores transposed so softmax sums come out of the matmul
  O^T   [512, S]   bf16 per-head outputs, feature-major, out-proj lhsT
"""

import math

import ml_dtypes
import numpy as np

import concourse.bass as bass
import concourse.bacc as bacc
import concourse.mybir as mybir
from concourse import tile
from concourse.bass_utils import run_bass_kernel_spmd

AF = mybir.ActivationFunctionType
ALU = mybir.AluOpType
F32 = mybir.dt.float32
F32R = mybir.dt.float32r
BF16 = mybir.dt.bfloat16
BF16_NP = ml_dtypes.bfloat16

N_HEADS = 16
THETA = 10000.0
D = 1024
HD = 64
HL = 8          # heads per core
VW = HD + 1     # v columns per head (64 dims + ones)
NB = 512        # stage-1 token block
QB = 512        # query block
TT = 128        # key/value tile


def _host_constants(S):
    """RoPE tables, signed-swap permutation, causal mask (input-independent)."""
    half = HD // 2
    inv = 1.0 / (THETA ** (np.arange(half, dtype=np.float64) / half))
    t = np.arange(S, dtype=np.float64)
    ang = inv[:, None] * t[None, :]                      # [32, S]
    ropeC = np.tile(np.cos(ang), (4, 1)).astype(BF16_NP)   # [128, S]
    sinT = np.tile(np.sin(ang), (4, 1)).astype(BF16_NP)    # [128, S]

    # perm[k, m] = sig(m) iff k == swap(m): out[m] = sig(m) * in[swap(m)]
    perm = np.zeros((128, 128), dtype=BF16_NP)
    for m in range(128):
        blk, inner = (m // HD) * HD, m % HD
        partner = blk + (inner + half) % HD
        perm[partner, m] = -1.0 if inner < half else 1.0

    # maskT2[p, c] over two 128-col copies: upper-triangular keep (c >= p)
    p = np.arange(TT)[:, None]
    c = np.arange(TT)[None, :]
    m1 = (c >= p).astype(BF16_NP)
    maskT2 = np.concatenate([m1, m1], axis=1)            # [128, 256]
    return ropeC, sinT, perm, maskT2


def build_nc(S=2048):
    nc = bacc.Bacc("TRN2", target_bir_lowering=False, debug=False)

    xT = nc.dram_tensor("xT", [D, S], BF16, kind="ExternalInput").ap()
    wqkT = nc.dram_tensor("wqkT", [D, 2 * HL * HD], BF16, kind="ExternalInput").ap()
    wvT = nc.dram_tensor("wvT", [D, HL * HD], BF16, kind="ExternalInput").ap()
    woutT = nc.dram_tensor("woutT", [HL * HD, D], BF16, kind="ExternalInput").ap()
    outp = nc.dram_tensor("outp", [S, D], mybir.dt.float16, kind="ExternalOutput").ap()

    ropeC_np, sinT_np, perm_np, maskT2_np = _host_constants(S)
    ropeC_d = nc.inline_tensor(ropeC_np, "ropeC").ap()
    sinT_d = nc.inline_tensor(sinT_np, "sinT").ap()
    perm_d = nc.inline_tensor(perm_np, "perm").ap()
    maskT2_d = nc.inline_tensor(maskT2_np, "maskT2").ap()
    onesbc_d = nc.inline_tensor(np.ones((128, HD), dtype=BF16_NP), "onesbc").ap()

    KD = D // 128        # 8 contraction tiles
    nNB = S // NB        # 4
    nQB = S // QB        # 4
    NTPB = QB // TT      # 4
    scale = 1.0 / math.sqrt(HD)

    with tile.TileContext(nc) as tc:
        with (
            tc.tile_pool(name="qk", bufs=1) as qk_pool,
            tc.tile_pool(name="vres", bufs=1) as v_pool,
            tc.tile_pool(name="osb", bufs=1) as o_pool,
            tc.tile_pool(name="wqk", bufs=1) as wqk_pool,
            tc.tile_pool(name="wv", bufs=1) as wv_pool,
            tc.tile_pool(name="wout", bufs=1) as wout_pool,
            tc.tile_pool(name="tabs", bufs=1) as tab_pool,
            tc.tile_pool(name="xs", bufs=2) as x_pool,
            tc.tile_pool(name="t1p", bufs=4) as t1_pool,
            tc.tile_pool(name="ptp", bufs=4) as pt_pool,
            tc.tile_pool(name="recp", bufs=2) as rec_pool,
            tc.tile_pool(name="t2p", bufs=2) as t2_pool,
            tc.tile_pool(name="otp", bufs=3) as ot_pool,
            tc.tile_pool(name="psA", bufs=2, space="PSUM") as psA,
            tc.tile_pool(name="psB", bufs=2, space="PSUM") as psB,
            tc.tile_pool(name="psO", bufs=2, space="PSUM") as psO,
        ):
            qk_sb = [qk_pool.tile([128, S], BF16, tag=f"qk{i}", name=f"qk{i}")
                     for i in range(8)]
            v_sb = [v_pool.tile([128, HL * VW], BF16, tag=f"v{i}", name=f"v{i}")
                    for i in range(S // TT)]
            o_sb = [o_pool.tile([128, S], BF16, tag=f"o{i}", name=f"o{i}")
                    for i in range(4)]
            wqkb = wqk_pool.tile([128, KD * 2 * HL * HD], BF16, tag="wqkb", name="wqkb")
            wvb = wv_pool.tile([128, KD * HL * HD], BF16, tag="wvb", name="wvb")
            wob = wout_pool.tile([128, 4 * D], BF16, tag="wob", name="wob")
            wqk_sb = [wqkb[:, k * 2 * HL * HD:(k + 1) * 2 * HL * HD] for k in range(KD)]
            wv_sb = [wvb[:, k * HL * HD:(k + 1) * HL * HD] for k in range(KD)]
            wout_sb = [wob[:, i * D:(i + 1) * D] for i in range(4)]
            ropeC_sb = tab_pool.tile([128, S], BF16, tag="ropeC", name="ropeC")
            sinT_sb = tab_pool.tile([128, S], BF16, tag="sinT", name="sinT")
            perm_sb = tab_pool.tile([128, 128], BF16, tag="perm", name="perm")
            maskT2_sb = tab_pool.tile([128, 2 * TT], BF16, tag="maskT2", name="maskT2")
            onesbc_sb = tab_pool.tile([128, HD], BF16, tag="onesbc", name="onesbc")

            xts = {}   # nb -> list of 8 [128, NB] f32r column slices

            xT_r = xT.rearrange("(k p) c -> p k c", k=KD)

            def load_x(nb, queue=nc.sync, pieces=1):
                tk = slice(nb * NB, (nb + 1) * NB)
                xb = x_pool.tile([128, KD * NB], BF16, name="xb")
                xb_r = xb[:].rearrange("p (k c) -> p k c", k=KD)
                kstep = KD // pieces
                for i in range(pieces):
                    ksl = slice(i * kstep, (i + 1) * kstep)
                    queue.dma_start(xb_r[:, ksl, :], xT_r[:, ksl, tk])
                xts[nb] = [xb[:, k * NB:(k + 1) * NB] for k in range(KD)]

            # ---------------- preamble: weights + first x block ----------
            # split across issue queues so the first matmul group starts fast
            nc.scalar.dma_start(ropeC_sb[:], ropeC_d[:])
            nc.scalar.dma_start(sinT_sb[:], sinT_d[:])
            nc.scalar.dma_start(perm_sb[:], perm_d[:])
            wqkT_r = wqkT.rearrange("(k p) m -> p k m", k=KD)
            wqkb_r = wqkb[:].rearrange("p (k m) -> p k m", k=KD)
            wvT_r = wvT.rearrange("(k p) m -> p k m", k=KD)
            wvb_r = wvb[:].rearrange("p (k m) -> p k m", k=KD)
            woutT_r = woutT.rearrange("(i p) m -> p i m", i=4)
            wob_r = wob[:].rearrange("p (i m) -> p i m", i=4)
            # interleave x / wqk pieces so the first k-chain starts early
            xb0 = x_pool.tile([128, KD * NB], BF16, name="xb")
            xb0_r = xb0[:].rearrange("p (k c) -> p k c", k=KD)
            for k0, k1 in ((0, 1), (1, 2), (2, 4), (4, 8)):
                ksl = slice(k0, k1)
                nc.sync.dma_start(xb0_r[:, ksl, :], xT_r[:, ksl, 0:NB])
                nc.sync.dma_start(wqkb_r[:, ksl, :], wqkT_r[:, ksl, :])
            xts[0] = [xb0[:, k * NB:(k + 1) * NB] for k in range(KD)]
            nc.scalar.dma_start(wvb_r[:], wvT_r[:])
            nc.scalar.dma_start(maskT2_sb[:], maskT2_d[:])
            nc.scalar.dma_start(onesbc_sb[:], onesbc_d[:])
            nc.scalar.dma_start(wob_r[:], woutT_r[:])

            # ---------------- stage 1: qkv projection + RoPE --------------
            def s1_rope(nb, mo, ps_ap, on_act=True):
                tok = slice(nb * NB, (nb + 1) * NB)
                dst = qk_sb[mo][:, tok]
                qs = psA.tile([128, NB], F32, tag="psA", name="qs")
                if on_act:
                    # PSUM read on ACT; bf16 SBUF DVE ops run in 2x mode
                    praw = t1_pool.tile([128, NB], BF16, tag="t1", name="praw")
                    nc.scalar.copy(praw[:], ps_ap)
                    nc.tensor.matmul(qs[:], lhsT=perm_sb[:], rhs=praw[:],
                                     start=True, stop=True)
                    nc.vector.tensor_tensor(dst, praw[:], ropeC_sb[:, tok],
                                            op=ALU.mult)
                    tm = t1_pool.tile([128, NB], BF16, tag="t1", name="tm")
                    nc.vector.tensor_tensor(tm[:], qs[:], sinT_sb[:, tok],
                                            op=ALU.mult)
                    nc.vector.tensor_tensor(dst, dst, tm[:], op=ALU.add)
                else:
                    # DVE-only variant (sin table is swap-invariant)
                    t1 = t1_pool.tile([128, NB], BF16, tag="t1", name="t1")
                    nc.vector.tensor_tensor(t1[:], ps_ap, sinT_sb[:, tok],
                                            op=ALU.mult)
                    nc.tensor.matmul(qs[:], lhsT=perm_sb[:], rhs=t1[:],
                                     start=True, stop=True)
                    nc.vector.tensor_tensor(dst, ps_ap, ropeC_sb[:, tok],
                                            op=ALU.mult)
                    nc.vector.tensor_tensor(dst, dst, qs[:], op=ALU.add)

            def s1_qk(nb, mo):
                ps = psA.tile([128, NB], F32, tag="psA", name="ps")
                for k in range(KD):
                    nc.tensor.matmul(
                        ps[:],
                        lhsT=wqk_sb[k][:, mo * 128:(mo + 1) * 128],
                        rhs=xts[nb][k][:],
                        start=(k == 0), stop=(k == KD - 1),
                    )
                s1_rope(nb, mo, ps[:], on_act=(mo % 2 == 0))

            def s1_v_mm(nb, mt, pv_ap, ks):
                xsl = slice(mt * 128, (mt + 1) * 128)
                for k in ks:
                    nc.tensor.matmul(
                        pv_ap,
                        lhsT=xts[nb][k][:, xsl],
                        rhs=wv_sb[k],
                        start=(k == 0), stop=(k == KD - 1),
                    )

            def s1_v_fin(nb, mt, pv, on_act=False):
                vt = v_sb[nb * (NB // TT) + mt]
                vre = vt[:].rearrange("p (h c) -> p h c", h=HL)
                if on_act:
                    nc.scalar.copy(
                        vre[:, :, 0:HD], pv[:].rearrange("p (h c) -> p h c", h=HL))
                else:
                    nc.vector.tensor_copy(
                        vre[:, :, 0:HD], pv[:].rearrange("p (h c) -> p h c", h=HL))
                nc.vector.memset(vre[:, :, HD], 1.0)

            def s1_v(nb, mt, on_act=False):
                pv = psA.tile([128, HL * HD], F32, tag="psA", name="pv")
                s1_v_mm(nb, mt, pv[:], range(KD))
                s1_v_fin(nb, mt, pv, on_act=on_act)

            # ---------------- stage 2: causal attention ------------------
            def s2_scores(qb, hp, j, pts):
                qt = qk_sb[hp]
                kt = qk_sb[4 + hp]
                oi = j - NTPB * qb
                c0 = max(oi, 0) * TT
                st = psB.tile([128, 2 * QB], F32, tag="psB", name="st")
                for hi in (0, 1):
                    base = hi * HD
                    nc.tensor.matmul(
                        st[:, hi * QB + c0:(hi + 1) * QB],
                        lhsT=kt[base:base + HD, j * TT:(j + 1) * TT],
                        rhs=qt[base:base + HD, qb * QB + c0:(qb + 1) * QB],
                        start=True, stop=True,
                    )
                pt = pt_pool.tile([128, 2 * QB], BF16, name="pt")
                st2 = st[:].rearrange("p (h c) -> p h c", h=2)
                pt2 = pt[:].rearrange("p (h c) -> p h c", h=2)
                nc.scalar.activation(pt2[:, :, c0:QB], st2[:, :, c0:QB],
                                     AF.Exp, scale=scale)
                if oi >= 0:
                    for hi in (0, 1):
                        csl = slice(hi * QB + c0, hi * QB + c0 + TT)
                        nc.vector.tensor_tensor(
                            pt[:, csl], pt[:, csl],
                            maskT2_sb[:, hi * TT:(hi + 1) * TT],
                            op=ALU.mult)
                pts[j] = pt

            def s2_pv(qb, hp, j, jmax, pts, oaug):
                h0, h1 = 2 * hp, 2 * hp + 1
                oi = j - NTPB * qb
                c0 = max(oi, 0) * TT
                pt = pts.pop(j)
                for hi, hh in ((0, h0), (1, h1)):
                    nc.tensor.matmul(
                        oaug[hi][:, c0:QB],
                        lhsT=v_sb[j][:, VW * hh:VW * hh + VW],
                        rhs=pt[:, hi * QB + c0:(hi + 1) * QB],
                        start=(j == 0), stop=(j == jmax - 1),
                    )

            def s2_tail(qb, hp, oaug):
                # reciprocal of denominators (partition 64), PE outer-product
                # broadcast, then scale+extract per head
                qsl = slice(qb * QB, (qb + 1) * QB)
                rec = rec_pool.tile([128, 2 * QB], BF16, name="rec")
                with nc.allow_low_precision(reason="softmax denom in bf16"):
                    for hi in (0, 1):
                        nc.vector.reciprocal(
                            rec[HD:VW, hi * QB:(hi + 1) * QB],
                            oaug[hi][HD:VW, :])
                bcs = []
                for hi in (0, 1):
                    bc = psA.tile([HD, QB], F32, tag="psA", name="bc")
                    nc.tensor.matmul(
                        bc[:],
                        lhsT=onesbc_sb[HD:VW, :],
                        rhs=rec[HD:VW, hi * QB:(hi + 1) * QB],
                        start=True, stop=True,
                    )
                    bcsb = t2_pool.tile([HD, QB], BF16, tag="bcsb", name="bcsb")
                    nc.scalar.copy(bcsb[:], bc[:])
                    bcs.append(bcsb)
                nc.vector.tensor_tensor(
                    o_sb[hp][0:HD, qsl], oaug[0][0:HD, :], bcs[0][:],
                    op=ALU.mult)
                t2 = t2_pool.tile([HD, QB], BF16, name="t2")
                nc.vector.tensor_tensor(t2[:], oaug[1][0:HD, :], bcs[1][:],
                                        op=ALU.mult)
                nc.sync.dma_start(o_sb[hp][HD:128, qsl], t2[:])

            def run_qb(qb, fillers, slots=None, late=()):
                """Emit one qb phase: lag-2 PV issue + filler interleave."""
                jmax = NTPB * (qb + 1)
                total = 4 * jmax
                if slots is None:
                    slots = [int(round((i + 1) * total / (len(fillers) + 1)))
                             for i in range(len(fillers))]
                fi = 0
                slot = 0
                for hp in range(4):
                    oaug = [psO.tile([VW, QB], F32, tag="psO", name="oaug")
                            for _ in range(2)]
                    pend = []
                    for j in range(jmax):
                        s2_scores(qb, hp, j, pend_pts)
                        pend.append(j)
                        if len(pend) > 2:
                            s2_pv(qb, hp, pend.pop(0), jmax, pend_pts, oaug)
                        slot += 1
                        while fi < len(fillers) and slots[fi] <= slot:
                            fillers[fi]()
                            fi += 1
                    while pend:
                        s2_pv(qb, hp, pend.pop(0), jmax, pend_pts, oaug)
                    if hp == 3:
                        for f in late:
                            f()
                    s2_tail(qb, hp, oaug)
                while fi < len(fillers):
                    fillers[fi]()
                    fi += 1

            pend_pts = {}

            # ---------------- stage 3: out projection ---------------------
            def s3_mt(qb, mt):
                    tsl = slice(qb * QB + mt * 128, qb * QB + (mt + 1) * 128)
                    ot = ot_pool.tile([128, D], mybir.dt.float16, name="ot")
                    for ib in range(2):
                        po = psB.tile([128, 512], F32, tag="psB", name="po")
                        for hp in range(4):
                            nc.tensor.matmul(
                                po[:],
                                lhsT=o_sb[hp][:, tsl],
                                rhs=wout_sb[hp][:, ib * 512:(ib + 1) * 512],
                                start=(hp == 0), stop=(hp == 3),
                            )
                        nc.vector.tensor_copy(ot[:, ib * 512:(ib + 1) * 512],
                                              po[:])
                    nc.sync.dma_start(outp[tsl, :], ot[:])

            # ---------------- pipelined emission --------------------------
            # nb=0: 8 concurrent groups (6 qk-mo on psA x2 + psB halves x4,
            # 2 v-tiles on psO), k-interleaved to match the x/wqk DMA pieces
            ps0 = psA.tile([128, NB], F32, tag="psA", name="ps")
            ps1 = psA.tile([128, NB], F32, tag="psA", name="ps")
            stb0 = psB.tile([128, 2 * QB], F32, tag="psB", name="st")
            stb1 = psB.tile([128, 2 * QB], F32, tag="psB", name="st")
            pv0 = psO.tile([128, HL * HD], F32, tag="psO", name="oaug")
            pv1 = psO.tile([128, HL * HD], F32, tag="psO", name="oaug")
            g_aps = [ps0[:], ps1[:], stb0[:, 0:512], stb0[:, 512:1024],
                     stb1[:, 0:512], stb1[:, 512:1024]]
            for ks in ((0, 1), (2, 3), (4, 5), (6, 7)):
                for g in range(6):
                    for k in ks:
                        nc.tensor.matmul(
                            g_aps[g],
                            lhsT=wqk_sb[k][:, g * 128:(g + 1) * 128],
                            rhs=xts[0][k],
                            start=(k == 0), stop=(k == KD - 1),
                        )
                s1_v_mm(0, 0, pv0[:], ks)
                s1_v_mm(0, 1, pv1[:], ks)
            # ropes interleaved with the remaining PE work (mo6/7, v2/3)
            s1_rope(0, 0, g_aps[0], on_act=True)
            s1_rope(0, 1, g_aps[1], on_act=True)
            s1_v_fin(0, 0, pv0, on_act=True)
            s1_v_fin(0, 1, pv1, on_act=True)
            s1_rope(0, 2, g_aps[2], on_act=True)
            s1_qk(0, 6)
            s1_rope(0, 3, g_aps[3], on_act=True)
            s1_qk(0, 7)
            s1_rope(0, 4, g_aps[4], on_act=True)
            s1_v(0, 2, on_act=True)
            s1_rope(0, 5, g_aps[5], on_act=True)
            s1_v(0, 3, on_act=True)
            load_x(1, pieces=2)
            # qb=0: overlap with full s1(1)
            f0 = [lambda m=m: s1_qk(1, m) for m in range(8)]
            f0 += [lambda m=m: s1_v(1, m) for m in range(4)]
            run_qb(0, f0)
            load_x(2, pieces=2)
            # qb=1: overlap with full s1(2) + s3(0)
            f1 = [lambda m=m: s1_qk(2, m) for m in range(8)]
            f1 += [lambda m=m: s1_v(2, m) for m in range(4)]
            f1 += [lambda m=m: s3_mt(0, m) for m in range(4)]
            run_qb(1, f1)
            load_x(3, pieces=2)
            # qb=2: overlap with s1(3) q-projection + s3(1)
            f2 = [lambda m=m: s1_qk(3, m) for m in range(4)]
            f2 += [lambda m=m: s3_mt(1, m) for m in range(4)]
            run_qb(2, f2)
            # qb=3: backfill with deferred s1(3) k-projection + v + s3(2)
            f3 = [lambda: s1_qk(3, 4)]
            f3 += [lambda m=m: s1_v(3, m) for m in range(4)]
            f3 += [lambda m=m: s1_qk(3, 4 + m) for m in (1, 2, 3)]
            f3 += [lambda m=m: s3_mt(2, m) for m in range(2)]
            # front-load k(hp0) + all v tiles so hp0's diagonal PVs are fed;
            # hold two s3(2) units back to cover the last tail chain
            run_qb(3, f3, slots=[1, 3, 5, 7, 9, 14, 22, 30, 42, 52],
                   late=[lambda m=m: s3_mt(2, m) for m in (2, 3)])
            for mt in range(4):
                s3_mt(3, mt)

    nc.compile()
    return nc


# ---------------------------------------------------------------------------
# host side
# ---------------------------------------------------------------------------

_cache = {}


def _get_nc(S):
    if S not in _cache:
        _cache[S] = build_nc(S)
    return _cache[S]


def _shard_weights(w_qkv, w_out, g):
    """Per-head-group weight shards in device layouts."""
    w_qkv = np.asarray(w_qkv, dtype=np.float32)
    w_out = np.asarray(w_out, dtype=np.float32)
    r = slice(g * 512, (g + 1) * 512)
    wq = w_qkv[0:D][r]            # [512, 1024]
    wk = w_qkv[D:2 * D][r]
    wv = w_qkv[2 * D:3 * D][r]
    wqkT = np.ascontiguousarray(np.concatenate([wq, wk], axis=0).T).astype(BF16_NP)
    wvT = np.ascontiguousarray(wv.T).astype(BF16_NP)                  # [1024, 512]
    woutT = np.ascontiguousarray(w_out.T[r]).astype(BF16_NP)          # [512, 1024]
    return wqkT, wvT, woutT


def kernel(x, w_qkv, w_out):
    x = np.asarray(x, dtype=np.float32)
    B, S, _D = x.shape
    assert _D == D
    nc = _get_nc(S)

    shards = [_shard_weights(w_qkv, w_out, g) for g in range(2)]
    in_maps = []
    for core in range(8):
        b, g = core // 2, core % 2
        wqkT, wvT, woutT = shards[g]
        in_maps.append({
            "xT": np.ascontiguousarray(x[b].T).astype(BF16_NP),
            "wqkT": wqkT,
            "wvT": wvT,
            "woutT": woutT,
        })
    res = run_bass_kernel_spmd(nc, in_maps, list(range(8)))
    out = np.empty((B, S, D), dtype=np.float32)
    for b in range(B):
        out[b] = (res.results[2 * b]["outp"].astype(np.float32)
                  + res.results[2 * b + 1]["outp"].astype(np.float32))
    return out


# revision 4
# speedup vs baseline: 1.2861x; 1.0438x over previous
"""Multi-head causal attention (B=4, S=2048, D=1024, H=16, RoPE) on 8 TRN2 cores.

Sharding: core = (batch b, head-group g of 8 heads).  Each core computes
qkv projection for its (b, g), RoPE, causal attention, and a partial
out-projection (contraction over its 512 head-dims).  Host sums the two
partials per batch.

v2 pipeline: qb-outer software pipeline interleaving stage1(nb+1) /
stage2(qb) / stage3(qb); bf16 data tiles; RoPE rotate-half via a PE
permutation matmul; softmax denominator broadcast via a PE outer
product; exact causal trapezoid (bf16 has no small-free-dim penalty).

Device layouts (per core):
  qk^T  [1024, S]  bf16 feature-major: tiles 0:4 = q (8 heads x 64), 4:8 = k
  v     [S, 520]   bf16 token-major, 65 cols/head: 64 dims + ones column
                   (accumulates softmax denominators during the P@V matmul)
  S^T   [t, q]     scores transposed so softmax sums come out of the matmul
  O^T   [512, S]   bf16 per-head outputs, feature-major, out-proj lhsT
"""

import math

import ml_dtypes
import numpy as np

import concourse.bass as bass
import concourse.bacc as bacc
import concourse.mybir as mybir
from concourse import tile
from concourse.bass_utils import run_bass_kernel_spmd

AF = mybir.ActivationFunctionType
ALU = mybir.AluOpType
F32 = mybir.dt.float32
F32R = mybir.dt.float32r
BF16 = mybir.dt.bfloat16
BF16_NP = ml_dtypes.bfloat16

N_HEADS = 16
THETA = 10000.0
D = 1024
HD = 64
HL = 8          # heads per core
VW = HD + 1     # v columns per head (64 dims + ones)
NB = 512        # stage-1 token block
QB = 512        # query block
TT = 128        # key/value tile


def _host_constants(S):
    """RoPE tables, signed-swap permutation, causal mask (input-independent)."""
    half = HD // 2
    inv = 1.0 / (THETA ** (np.arange(half, dtype=np.float64) / half))
    t = np.arange(S, dtype=np.float64)
    ang = inv[:, None] * t[None, :]                      # [32, S]
    ropeC = np.tile(np.cos(ang), (4, 1)).astype(BF16_NP)   # [128, S]
    sinT = np.tile(np.sin(ang), (4, 1)).astype(BF16_NP)    # [128, S]

    # perm[k, m] = sig(m) iff k == swap(m): out[m] = sig(m) * in[swap(m)]
    perm = np.zeros((128, 128), dtype=BF16_NP)
    for m in range(128):
        blk, inner = (m // HD) * HD, m % HD
        partner = blk + (inner + half) % HD
        perm[partner, m] = -1.0 if inner < half else 1.0

    # maskT2[p, c] over two 128-col copies: upper-triangular keep (c >= p)
    p = np.arange(TT)[:, None]
    c = np.arange(TT)[None, :]
    m1 = (c >= p).astype(BF16_NP)
    maskT2 = np.concatenate([m1, m1], axis=1)            # [128, 256]
    return ropeC, sinT, perm, maskT2


def build_nc(S=2048):
    nc = bacc.Bacc("TRN2", target_bir_lowering=False, debug=False)

    xT = nc.dram_tensor("xT", [D, S], BF16, kind="ExternalInput").ap()
    wqkT = nc.dram_tensor("wqkT", [D, 2 * HL * HD], BF16, kind="ExternalInput").ap()
    wvT = nc.dram_tensor("wvT", [D, HL * HD], BF16, kind="ExternalInput").ap()
    woutT = nc.dram_tensor("woutT", [HL * HD, D], BF16, kind="ExternalInput").ap()
    outp = nc.dram_tensor("outp", [S, D], mybir.dt.float16, kind="ExternalOutput").ap()

    ropeC_np, sinT_np, perm_np, maskT2_np = _host_constants(S)
    ropeC_d = nc.inline_tensor(ropeC_np, "ropeC").ap()
    sinT_d = nc.inline_tensor(sinT_np, "sinT").ap()
    perm_d = nc.inline_tensor(perm_np, "perm").ap()
    maskT2_d = nc.inline_tensor(maskT2_np, "maskT2").ap()
    ident_d = nc.inline_tensor(np.eye(128, dtype=BF16_NP), "ident").ap()

    KD = D // 128        # 8 contraction tiles
    nNB = S // NB        # 4
    nQB = S // QB        # 4
    NTPB = QB // TT      # 4
    scale = 1.0 / math.sqrt(HD)
    PVLAG = 0

    with tile.TileContext(nc) as tc:
        with (
            tc.tile_pool(name="qk", bufs=1) as qk_pool,
            tc.tile_pool(name="vres", bufs=1) as v_pool,
            tc.tile_pool(name="osb", bufs=1) as o_pool,
            tc.tile_pool(name="wqk", bufs=1) as wqk_pool,
            tc.tile_pool(name="wv", bufs=1) as wv_pool,
            tc.tile_pool(name="wout", bufs=1) as wout_pool,
            tc.tile_pool(name="tabs", bufs=1) as tab_pool,
            tc.tile_pool(name="xs", bufs=2) as x_pool,
            tc.tile_pool(name="t1p", bufs=6) as t1_pool,
            tc.tile_pool(name="ptp", bufs=18) as pt_pool,
            tc.tile_pool(name="recp", bufs=3) as rec_pool,
            tc.tile_pool(name="t2p", bufs=4) as t2_pool,
            tc.tile_pool(name="otp", bufs=4) as ot_pool,
            tc.tile_pool(name="psA", bufs=2, space="PSUM") as psA,
            tc.tile_pool(name="psB", bufs=2, space="PSUM") as psB,
            tc.tile_pool(name="psO", bufs=1, space="PSUM") as psO,
        ):
            qk_sb = [qk_pool.tile([128, S], BF16, tag=f"qk{i}", name=f"qk{i}")
                     for i in range(8)]
            v_sb = [v_pool.tile([128, HL * VW], BF16, tag=f"v{i}", name=f"v{i}")
                    for i in range(S // TT)]
            o_sb = [o_pool.tile([128, S], BF16, tag=f"o{i}", name=f"o{i}")
                    for i in range(4)]
            wqkb = wqk_pool.tile([128, KD * 2 * HL * HD], BF16, tag="wqkb", name="wqkb")
            wvb = wv_pool.tile([128, KD * HL * HD], BF16, tag="wvb", name="wvb")
            wob = wout_pool.tile([128, 4 * D], BF16, tag="wob", name="wob")
            wqk_sb = [wqkb[:, k * 2 * HL * HD:(k + 1) * 2 * HL * HD] for k in range(KD)]
            wv_sb = [wvb[:, k * HL * HD:(k + 1) * HL * HD] for k in range(KD)]
            wout_sb = [wob[:, i * D:(i + 1) * D] for i in range(4)]
            ropeC_sb = tab_pool.tile([128, S], BF16, tag="ropeC", name="ropeC")
            sinT_sb = tab_pool.tile([128, S], BF16, tag="sinT", name="sinT")
            perm_sb = tab_pool.tile([128, 128], BF16, tag="perm", name="perm")
            maskT2_sb = tab_pool.tile([128, 2 * TT], BF16, tag="maskT2", name="maskT2")
            ident_sb = tab_pool.tile([128, 128], BF16, tag="ident", name="ident")

            xts = {}   # nb -> list of 8 [128, NB] f32r column slices

            xT_r = xT.rearrange("(k p) c -> p k c", k=KD)

            def load_x(nb, queue=nc.sync, pieces=1):
                tk = slice(nb * NB, (nb + 1) * NB)
                xb = x_pool.tile([128, KD * NB], BF16, name="xb")
                xb_r = xb[:].rearrange("p (k c) -> p k c", k=KD)
                kstep = KD // pieces
                for i in range(pieces):
                    ksl = slice(i * kstep, (i + 1) * kstep)
                    queue.dma_start(xb_r[:, ksl, :], xT_r[:, ksl, tk])
                xts[nb] = [xb[:, k * NB:(k + 1) * NB] for k in range(KD)]

            # ---------------- preamble: weights + first x block ----------
            # split across issue queues so the first matmul group starts fast
            wqkT_r = wqkT.rearrange("(k p) m -> p k m", k=KD)
            wqkb_r = wqkb[:].rearrange("p (k m) -> p k m", k=KD)
            wvT_r = wvT.rearrange("(k p) m -> p k m", k=KD)
            wvb_r = wvb[:].rearrange("p (k m) -> p k m", k=KD)
            woutT_r = woutT.rearrange("(i p) m -> p i m", i=4)
            wob_r = wob[:].rearrange("p (i m) -> p i m", i=4)
            # interleave x / wqk pieces so the first k-chain starts early
            xb0 = x_pool.tile([128, KD * NB], BF16, name="xb")
            xb0_r = xb0[:].rearrange("p (k c) -> p k c", k=KD)
            nc.scalar.dma_start(ropeC_sb[:], ropeC_d[:])
            nc.scalar.dma_start(sinT_sb[:], sinT_d[:])
            nc.scalar.dma_start(perm_sb[:], perm_d[:])
            for k0, k1 in ((0, 1), (1, 2), (2, 4), (4, 8)):
                ksl = slice(k0, k1)
                nc.sync.dma_start(xb0_r[:, ksl, :], xT_r[:, ksl, 0:NB])
                nc.sync.dma_start(wqkb_r[:, ksl, :], wqkT_r[:, ksl, :])
            xts[0] = [xb0[:, k * NB:(k + 1) * NB] for k in range(KD)]
            nc.scalar.dma_start(wvb_r[:], wvT_r[:])
            nc.scalar.dma_start(maskT2_sb[:], maskT2_d[:])
            nc.scalar.dma_start(ident_sb[:], ident_d[:])
            nc.scalar.dma_start(wob_r[:], woutT_r[:])

            # ---------------- stage 1: qkv projection + RoPE --------------
            def s1_rope(nb, mo, ps_ap, on_act=True):
                tok = slice(nb * NB, (nb + 1) * NB)
                dst = qk_sb[mo][:, tok]
                qs = psA.tile([128, NB], F32, tag="psA", name="qs")
                if on_act:
                    # PSUM read on ACT; bf16 SBUF DVE ops run in 2x mode
                    praw = t1_pool.tile([128, NB], BF16, tag="t1", name="praw")
                    nc.scalar.copy(praw[:], ps_ap)
                    nc.tensor.matmul(qs[:], lhsT=perm_sb[:], rhs=praw[:],
                                     start=True, stop=True)
                    nc.vector.tensor_tensor(dst, praw[:], ropeC_sb[:, tok],
                                            op=ALU.mult)
                    tm = t1_pool.tile([128, NB], BF16, tag="t1", name="tm")
                    nc.vector.tensor_tensor(tm[:], qs[:], sinT_sb[:, tok],
                                            op=ALU.mult)
                    nc.vector.tensor_tensor(dst, dst, tm[:], op=ALU.add)
                else:
                    # DVE-only variant (sin table is swap-invariant)
                    t1 = t1_pool.tile([128, NB], BF16, tag="t1", name="t1")
                    nc.vector.tensor_tensor(t1[:], ps_ap, sinT_sb[:, tok],
                                            op=ALU.mult)
                    nc.tensor.matmul(qs[:], lhsT=perm_sb[:], rhs=t1[:],
                                     start=True, stop=True)
                    nc.vector.tensor_tensor(dst, ps_ap, ropeC_sb[:, tok],
                                            op=ALU.mult)
                    nc.vector.tensor_tensor(dst, dst, qs[:], op=ALU.add)

            _qk_ps = {}

            def s1_qk_h(nb, mo, half):
                if half == 0:
                    _qk_ps[(nb, mo)] = psA.tile([128, NB], F32, tag="psA",
                                                name="ps")
                ps = _qk_ps[(nb, mo)]
                for k in range(half * 4, half * 4 + 4):
                    nc.tensor.matmul(
                        ps[:],
                        lhsT=wqk_sb[k][:, mo * 128:(mo + 1) * 128],
                        rhs=xts[nb][k][:],
                        start=(k == 0), stop=(k == KD - 1),
                    )
                if half == 1:
                    del _qk_ps[(nb, mo)]
                    if nb == 1:
                        on_act = True
                    elif nb == 3:
                        on_act = False
                    else:
                        on_act = (mo % 2 == 0)
                    s1_rope(nb, mo, ps[:], on_act=on_act)

            def s1_qk(nb, mo):
                s1_qk_h(nb, mo, 0)
                s1_qk_h(nb, mo, 1)

            def s1_v_mm(nb, mt, pv_ap, ks):
                xsl = slice(mt * 128, (mt + 1) * 128)
                for k in ks:
                    nc.tensor.matmul(
                        pv_ap,
                        lhsT=xts[nb][k][:, xsl],
                        rhs=wv_sb[k],
                        start=(k == 0), stop=(k == KD - 1),
                    )

            def s1_v_fin(nb, mt, pv, on_act=False):
                vt = v_sb[nb * (NB // TT) + mt]
                vre = vt[:].rearrange("p (h c) -> p h c", h=HL)
                if on_act:
                    nc.scalar.copy(
                        vre[:, :, 0:HD], pv[:].rearrange("p (h c) -> p h c", h=HL))
                else:
                    nc.vector.tensor_copy(
                        vre[:, :, 0:HD], pv[:].rearrange("p (h c) -> p h c", h=HL))
                nc.vector.memset(vre[:, :, HD], 1.0)

            def s1_v(nb, mt, on_act=False):
                pv = psA.tile([128, HL * HD], F32, tag="psA", name="pv")
                s1_v_mm(nb, mt, pv[:], range(KD))
                s1_v_fin(nb, mt, pv, on_act=on_act)

            # ---------------- stage 2: causal attention ------------------
            def s2_scores(qb, hp, j, pts):
                qt = qk_sb[hp]
                kt = qk_sb[4 + hp]
                oi = j - NTPB * qb
                c0 = max(oi, 0) * TT
                st = psB.tile([128, 2 * QB], F32, tag="psB", name="st")
                for hi in (0, 1):
                    base = hi * HD
                    nc.tensor.matmul(
                        st[:, hi * QB + c0:(hi + 1) * QB],
                        lhsT=kt[base:base + HD, j * TT:(j + 1) * TT],
                        rhs=qt[base:base + HD, qb * QB + c0:(qb + 1) * QB],
                        start=True, stop=True,
                    )
                pt = pt_pool.tile([128, 2 * QB], BF16, name="pt")
                st2 = st[:].rearrange("p (h c) -> p h c", h=2)
                pt2 = pt[:].rearrange("p (h c) -> p h c", h=2)
                nc.scalar.activation(pt2[:, :, c0:QB], st2[:, :, c0:QB],
                                     AF.Exp, scale=scale)
                if oi >= 0:
                    for hi in (0, 1):
                        csl = slice(hi * QB + c0, hi * QB + c0 + TT)
                        nc.vector.tensor_tensor(
                            pt[:, csl], pt[:, csl],
                            maskT2_sb[:, hi * TT:(hi + 1) * TT],
                            op=ALU.mult)
                pts[j] = pt

            def s2_pv_group(qb, hp, sub, pts, oaug8, rec, o_qm):
                # full accumulation chain for q-subblock `sub` (both heads),
                # followed immediately by its reciprocal + normalize
                h0, h1 = 2 * hp, 2 * hp + 1
                jlast = NTPB * qb + sub
                for hi, hh in ((0, h0), (1, h1)):
                    for j in range(jlast + 1):
                        nc.tensor.matmul(
                            oaug8[hi][:, VW * sub:VW * sub + VW],
                            lhsT=pts[j][:, hi * QB + sub * TT:hi * QB + (sub + 1) * TT],
                            rhs=v_sb[j][:, VW * hh:VW * hh + VW],
                            start=(j == 0), stop=(j == jlast),
                        )
                for hi in (0, 1):
                    g = hi * NTPB + sub
                    nc.vector.reciprocal(
                        rec[:, g:g + 1],
                        oaug8[hi][:, VW * sub + HD:VW * sub + VW])
                    nc.vector.tensor_scalar_mul(
                        o_qm[:, sub * TT + hi * HD:sub * TT + hi * HD + HD],
                        oaug8[hi][:, VW * sub:VW * sub + HD],
                        rec[:, g:g + 1])

            def s2_tail(qb, hp, o_qm):
                # PE transpose back to dim-major + one copy out
                tp = psA.tile([128, QB], BF16, tag="psA", name="tp")
                for sub in range(NTPB):
                    nc.tensor.transpose(
                        tp[:, sub * TT:(sub + 1) * TT],
                        o_qm[:, sub * TT:(sub + 1) * TT], ident_sb[:])
                qsl = slice(qb * QB, (qb + 1) * QB)
                if hp % 2 == 0:
                    nc.scalar.copy(o_sb[hp][:, qsl], tp[:])
                else:
                    nc.vector.tensor_copy(o_sb[hp][:, qsl], tp[:])

            def run_qb(qb, fillers, slots=None, late=()):
                """Emit one qb phase: lag-2 PV issue + filler interleave."""
                jmax = NTPB * (qb + 1)
                total = 4 * jmax
                if slots is None:
                    slots = [int(round((i + 1) * total / (len(fillers) + 1)))
                             for i in range(len(fillers))]
                fi = 0
                slot = 0
                for hp in range(4):
                    oaug8 = [psO.tile([128, NTPB * VW], F32, tag="psO",
                                      name="oaug") for _ in range(2)]
                    rec = rec_pool.tile([128, 2 * NTPB], F32, tag="rec8",
                                        name="rec")
                    o_qm = t2_pool.tile([128, QB], BF16, tag="t2", name="oqm")
                    for j in range(jmax):
                        s2_scores(qb, hp, j, pend_pts)
                        oi = j - NTPB * qb - PVLAG
                        if oi >= 0:
                            s2_pv_group(qb, hp, oi, pend_pts, oaug8, rec, o_qm)
                        slot += 1
                        while fi < len(fillers) and slots[fi] <= slot:
                            fillers[fi]()
                            fi += 1
                    for oi in range(NTPB - PVLAG, NTPB):
                        s2_pv_group(qb, hp, oi, pend_pts, oaug8, rec, o_qm)
                    pend_pts.clear()
                    s2_tail(qb, hp, o_qm)
                    if hp == 3:
                        for f in late:
                            f()
                while fi < len(fillers):
                    fillers[fi]()
                    fi += 1

            pend_pts = {}

            # ---------------- stage 3: out projection ---------------------
            _s3_ot = {}

            def s3_ib(qb, mt, ib, alt=False):
                    tsl = slice(qb * QB + mt * 128, qb * QB + (mt + 1) * 128)
                    if ib == 0:
                        _s3_ot[(qb, mt)] = ot_pool.tile(
                            [128, D], mybir.dt.float16, name="ot")
                    ot = _s3_ot[(qb, mt)]
                    po = psB.tile([128, 512], F32, tag="psB", name="po")
                    for hp in range(4):
                        nc.tensor.matmul(
                            po[:],
                            lhsT=o_sb[hp][:, tsl],
                            rhs=wout_sb[hp][:, ib * 512:(ib + 1) * 512],
                            start=(hp == 0), stop=(hp == 3),
                        )
                    osl = slice(ib * 512, (ib + 1) * 512)
                    if alt and ib == 0:
                        nc.scalar.copy(ot[:, osl], po[:])
                    else:
                        nc.vector.tensor_copy(ot[:, osl], po[:])
                    if ib == 1:
                        del _s3_ot[(qb, mt)]
                        nc.sync.dma_start(outp[tsl, :], ot[:])

            def s3_mt(qb, mt, alt=False):
                    s3_ib(qb, mt, 0, alt)
                    s3_ib(qb, mt, 1, alt)

            # ---------------- pipelined emission --------------------------
            # nb=0: 8 concurrent groups (6 qk-mo on psA x2 + psB halves x4,
            # 2 v-tiles on psO), k-interleaved to match the x/wqk DMA pieces
            ps0 = psA.tile([128, NB], F32, tag="psA", name="ps")
            ps1 = psA.tile([128, NB], F32, tag="psA", name="ps")
            stb0 = psB.tile([128, 2 * QB], F32, tag="psB", name="st")
            stb1 = psB.tile([128, 2 * QB], F32, tag="psB", name="st")
            pv0 = psO.tile([128, HL * HD], F32, tag="psO", name="oaug")
            g_aps = [ps0[:], ps1[:], stb0[:, 0:512], stb0[:, 512:1024],
                     stb1[:, 0:512], stb1[:, 512:1024]]
            for ks in ((0, 1), (2, 3), (4, 5), (6, 7)):
                for g in range(6):
                    for k in ks:
                        nc.tensor.matmul(
                            g_aps[g],
                            lhsT=wqk_sb[k][:, g * 128:(g + 1) * 128],
                            rhs=xts[0][k],
                            start=(k == 0), stop=(k == KD - 1),
                        )
                s1_v_mm(0, 0, pv0[:], ks)
            # ropes interleaved with the remaining PE work (mo6/7, v2/3)
            s1_rope(0, 0, g_aps[0], on_act=True)
            s1_rope(0, 1, g_aps[1], on_act=True)
            s1_v_fin(0, 0, pv0, on_act=True)
            s1_rope(0, 2, g_aps[2], on_act=True)
            s1_qk(0, 6)
            s1_rope(0, 3, g_aps[3], on_act=True)
            s1_qk(0, 7)
            s1_rope(0, 4, g_aps[4], on_act=True)
            s1_v(0, 1, on_act=True)
            s1_rope(0, 5, g_aps[5], on_act=True)
            s1_v(0, 2, on_act=True)
            s1_v(0, 3, on_act=True)
            load_x(1, pieces=2)
            # qb=0: overlap with full s1(1)
            f0 = []
            for m in range(8):
                f0 += [lambda m=m: s1_qk_h(1, m, 0), lambda m=m: s1_qk_h(1, m, 1)]
                if m % 2 == 1:
                    f0.append(lambda m=m: s1_v(1, m // 2))
            run_qb(0, f0)
            load_x(2, pieces=2)
            # qb=1: overlap with full s1(2) + s3(0)
            f1 = []
            for m in range(8):
                f1 += [lambda m=m: s1_qk_h(2, m, 0), lambda m=m: s1_qk_h(2, m, 1)]
                if m % 2 == 1:
                    f1.append(lambda m=m: s1_v(2, m // 2))
                    f1.append(lambda m=m: s3_ib(0, m // 2, 0))
                    f1.append(lambda m=m: s3_ib(0, m // 2, 1))
            run_qb(1, f1)
            load_x(3, pieces=2)
            # qb=2: overlap with s1(3) q-projection + s3(1)
            f2 = []
            for m in range(4):
                f2 += [lambda m=m: s1_qk_h(3, m, 0), lambda m=m: s1_qk_h(3, m, 1)]
                f2 += [lambda m=m: s3_ib(1, m, 0), lambda m=m: s3_ib(1, m, 1)]
            run_qb(2, f2)
            # qb=3: backfill with deferred s1(3) k-projection + v + s3(2)
            f3 = [lambda: s1_qk_h(3, 4, 0), lambda: s1_qk_h(3, 4, 1)]
            f3 += [lambda m=m: s1_v(3, m) for m in range(4)]
            for m in (1, 2, 3):
                f3 += [lambda m=m: s1_qk_h(3, 4 + m, 0),
                       lambda m=m: s1_qk_h(3, 4 + m, 1)]
            f3 += [lambda m=m: s3_ib(2, m // 2, m % 2) for m in range(4)]
            # front-load k(hp0) + all v tiles so hp0's diagonal PVs are fed;
            # hold two s3(2) units back to cover the last tail chain
            run_qb(3, f3, slots=[1, 2, 4, 5, 7, 8, 12, 16, 20, 26, 32, 38,
                                 44, 50, 54, 58],
                   late=[lambda m=m: s3_mt(2, m) for m in (2, 3)])
            for mt in range(4):
                s3_mt(3, mt, alt=True)

    nc.compile()
    return nc


# ---------------------------------------------------------------------------
# host side
# ---------------------------------------------------------------------------

_cache = {}


def _get_nc(S):
    if S not in _cache:
        _cache[S] = build_nc(S)
    return _cache[S]


def _shard_weights(w_qkv, w_out, g):
    """Per-head-group weight shards in device layouts."""
    w_qkv = np.asarray(w_qkv, dtype=np.float32)
    w_out = np.asarray(w_out, dtype=np.float32)
    r = slice(g * 512, (g + 1) * 512)
    wq = w_qkv[0:D][r]            # [512, 1024]
    wk = w_qkv[D:2 * D][r]
    wv = w_qkv[2 * D:3 * D][r]
    wqkT = np.ascontiguousarray(np.concatenate([wq, wk], axis=0).T).astype(BF16_NP)
    wvT = np.ascontiguousarray(wv.T).astype(BF16_NP)                  # [1024, 512]
    woutT = np.ascontiguousarray(w_out.T[r]).astype(BF16_NP)          # [512, 1024]
    return wqkT, wvT, woutT


def kernel(x, w_qkv, w_out):
    x = np.asarray(x, dtype=np.float32)
    B, S, _D = x.shape
    assert _D == D
    nc = _get_nc(S)

    shards = [_shard_weights(w_qkv, w_out, g) for g in range(2)]
    in_maps = []
    for core in range(8):
        b, g = core // 2, core % 2
        wqkT, wvT, woutT = shards[g]
        in_maps.append({
            "xT": np.ascontiguousarray(x[b].T).astype(BF16_NP),
            "wqkT": wqkT,
            "wvT": wvT,
            "woutT": woutT,
        })
    res = run_bass_kernel_spmd(nc, in_maps, list(range(8)))
    out = np.empty((B, S, D), dtype=np.float32)
    for b in range(B):
        out[b] = (res.results[2 * b]["outp"].astype(np.float32)
                  + res.results[2 * b + 1]["outp"].astype(np.float32))
    return out
